# revision 7
# baseline (speedup 1.0000x reference)
"""DiagSSMBlock Trainium2 kernel.

h_t = sum_{k=0..t} a^k * (B^T x_{t-k})  ==  h_t = a * h_{t-1} + s_t, s = B^T x^T.

Strategy: shard T across the 8 cores (1024 steps each + 8-step halo; |a| <=
sqrt(2/1024) ~ 0.044 so a^8 ~ 1.5e-11 — invisible at fp32, making slabs
independent).  Host passes x pre-transposed ([H, T_slab]) pre-cast to bf16,
with the per-channel `a` vector folded into its first 8 columns (saves a
whole DMA: each dma_start costs ~0.6 us issue + ~2.1 us serialized
completion on its ring).  bf16 keeps the PE at 1 cycle/row while halving
HBM traffic.

Layout (driven by measured ntff traces):
  - ~7 us framework preamble; N_WARM dummy matmuls bridge preamble-end to
    first-data so the PE HAM clock-gate is at 2.4 GHz for the real stream;
  - first-needed tiles are DMA #1 on each ring: (x0+av) on scalar, b0 on
    sync; remaining b rows follow on both rings in consumption order;
    x0-rest and x chunk-1/2 ride the SWDGE (gpsimd) ring whose completion
    latency is independent of the HWDGE rings;
  - chunk ni=0 runs kq-MAJOR across all 8 PSUM banks (one per 128-channel
    group) so compute starts as soon as (x0, b0) land and consumes b rows
    as they arrive; chunks 1-2 run g-major so each group's DVE scan frees
    its PSUM bank just ahead of reuse;
  - the SSM recurrence is DVE tensor_tensor_scan (1x only — no 2x uop
    exists for scans), reading PSUM, `a` via stride-0 broadcast AP, bf16
    output; two dummy probe scans with SBUF operands run during the idle
    early window to measure whether the 2.45 cyc/elem scan cost is a
    PSUM-read penalty;
  - stores alternate rings in scan-completion order.
"""

import sys

if "/opt/trn_rl_repo" not in sys.path:
    sys.path.insert(0, "/opt/trn_rl_repo")

import ml_dtypes
import numpy as np

T, H = 8192, 1024
NC = 8
P = 128
T_LOC = T // NC            # 1024 output timesteps per core
HALO = 8                   # scan warmup; a^8 ~ 1.5e-11
W = T_LOC + HALO           # 1032
AV = 8                     # a-vector columns folded into xt
CH = 344                   # psum chunk width (3 chunks of 344 = 1032)
NCHUNK = W // CH           # 3
KQ = H // P                # 8 contraction chunks
G = H // P                 # 8 output-channel groups
N_WARM = 28                # dummy matmuls to lift the HAM clock gate

_state = {}


def _build_nc():
    import concourse.tile as tile
    from concourse import bacc, mybir

    bf16 = mybir.dt.bfloat16
    f32 = mybir.dt.float32

    nc = bacc.Bacc("TRN2", target_bir_lowering=False, debug=False, num_devices=NC)
    xt_e = nc.dram_tensor("xt", [H, AV + W], bf16, kind="ExternalInput").ap()
    b_e = nc.dram_tensor("b", [H, H], bf16, kind="ExternalInput").ap()
    out_e = nc.dram_tensor("out", [H, T_LOC], bf16, kind="ExternalOutput").ap()
    flush_e = nc.dram_tensor("warm_flush", [P, 1], f32).ap()

    # kq-indexed views: row (q*128 + p) -> [p, q, col]
    xt_r = xt_e.rearrange("(q p) w -> p q w", p=P)
    b_r = b_e.rearrange("(q p) c -> p q c", p=P)

    with tile.TileContext(nc) as tc:
        with (
            tc.tile_pool(name="consts", bufs=1) as consts,
            tc.tile_pool(name="bpool", bufs=1) as bpool,
            tc.tile_pool(name="xpool", bufs=1) as xpool,
            tc.tile_pool(name="hpool", bufs=1) as hpool,
            tc.tile_pool(name="pspool", bufs=8, space="PSUM") as pspool,
        ):
            # PE warm-up: bridge preamble-end -> first-data with dummy MMs.
            warm_sb = consts.tile([P, P], bf16, tag="warm")
            nc.gpsimd.memset(warm_sb[:], 0.0)
            wps = pspool.tile([P, P], f32, tag="ps", name="wps")
            for i in range(N_WARM):
                nc.tensor.matmul(
                    wps[:],
                    warm_sb[:],
                    warm_sb[:],
                    start=(i == 0),
                    stop=(i == N_WARM - 1),
                )
            flush_sb = consts.tile([P, 1], f32, tag="flush")
            nc.vector.tensor_copy(flush_sb[:], wps[:, 0:1])

            # ---- input loads
            # scalar: x0_first(+av), b1, b67          | + stores (even g)
            # sync:   b0, b23, b45                    | + stores (odd g)
            # gpsimd (SWDGE): x0_rest, x12_all
            x0_first = xpool.tile([P, AV + CH], bf16, tag="x0f", name="x0_first")
            nc.scalar.dma_start(x0_first[:], xt_e[0:P, 0 : AV + CH])
            b0 = bpool.tile([P, H], bf16, tag="b0", name="b0")
            nc.sync.dma_start(b0[:], b_e[0:P, :])

            b1 = bpool.tile([P, H], bf16, tag="b1", name="b1")
            nc.scalar.dma_start(b1[:], b_e[P : 2 * P, :])
            b23 = bpool.tile([P, 2 * H], bf16, tag="b23", name="b23")
            nc.sync.dma_start(b23[:], b_r[:, 2:4, :])
            b67 = bpool.tile([P, 2 * H], bf16, tag="b67", name="b67")
            nc.scalar.dma_start(b67[:], b_r[:, 6:8, :])
            b45 = bpool.tile([P, 2 * H], bf16, tag="b45", name="b45")
            nc.sync.dma_start(b45[:], b_r[:, 4:6, :])

            x0_rest = xpool.tile([P, (KQ - 1) * CH], bf16, tag="x0r", name="x0_rest")
            nc.gpsimd.dma_start(x0_rest[:], xt_r[:, 1:KQ, AV : AV + CH])
            x12_all = xpool.tile([P, KQ * (W - CH)], bf16, tag="x12", name="x12_all")
            nc.gpsimd.dma_start(x12_all[:], xt_r[:, :, AV + CH : AV + W])

            a_op = [
                x0_first[:, g : g + 1].broadcast_to([P, CH]) for g in range(G)
            ]

            def b_slice(kq, g):
                if kq == 0:
                    return b0[:, g * P : (g + 1) * P]
                if kq == 1:
                    return b1[:, g * P : (g + 1) * P]
                pair, off = divmod(kq - 2, 2)
                t = (b23, b45, b67)[pair]
                return t[:, off * H + g * P : off * H + (g + 1) * P]

            def x_chunk(kq, ni):
                if ni == 0:
                    if kq == 0:
                        return x0_first[:, AV : AV + CH]
                    return x0_rest[:, (kq - 1) * CH : kq * CH]
                base = kq * (W - CH) + (ni - 1) * CH
                return x12_all[:, base : base + CH]

            # DVE probe scans (timing experiment, results unused): does a
            # scan with SBUF data1 run faster than the 2.45 cyc/elem
            # PSUM-read scans?  Runs in the otherwise-idle early window.
            scr_f = consts.tile([P, CH], f32, tag="scrf")
            scr_o = consts.tile([P, CH], bf16, tag="scro")
            nc.vector.memset(scr_f[:], 0.25)
            nc.vector.tensor_tensor_scan(
                scr_o[:], a_op[0], scr_f[:], 0.0,
                op0=mybir.AluOpType.mult, op1=mybir.AluOpType.add,
            )
            nc.vector.tensor_tensor_scan(
                scr_o[:], a_op[0], x0_rest[:, 0:CH], 0.0,
                op0=mybir.AluOpType.mult, op1=mybir.AluOpType.add,
            )

            h_t = [
                hpool.tile([P, W], bf16, tag=f"h{g}", name=f"h{g}")
                for g in range(G)
            ]

            def scan_and_store(g, ni, ps_g):
                n0 = ni * CH
                init = 0.0 if ni == 0 else h_t[g][:, n0 - 1 : n0]
                nc.vector.tensor_tensor_scan(
                    h_t[g][:, n0 : n0 + CH],
                    a_op[g],
                    ps_g[:],
                    init,
                    op0=mybir.AluOpType.mult,
                    op1=mybir.AluOpType.add,
                )
                lo = HALO if ni == 0 else 0
                eng = nc.scalar if g % 2 == 0 else nc.sync
                eng.dma_start(
                    out_e[g * P : (g + 1) * P, n0 + lo - HALO : n0 + CH - HALO],
                    h_t[g][:, n0 + lo : n0 + CH],
                )

            # chunk 0: kq-major across all 8 PSUM banks (DMA-arrival matched)
            ps0 = [
                pspool.tile([P, CH], f32, tag="ps", name=f"ps0_{g}")
                for g in range(G)
            ]
            for kq in range(KQ):
                for g in range(G):
                    nc.tensor.matmul(
                        ps0[g][:],
                        b_slice(kq, g),
                        x_chunk(kq, 0),
                        start=(kq == 0),
                        stop=(kq == KQ - 1),
                    )
            for g in range(G):
                scan_and_store(g, 0, ps0[g])

            # chunks 1-2: g-major, scans chase and free banks just in time
            for ni in (1, 2):
                for g in range(G):
                    ps_g = pspool.tile([P, CH], f32, tag="ps", name=f"ps{ni}_{g}")
                    for kq in range(KQ):
                        nc.tensor.matmul(
                            ps_g[:],
                            b_slice(kq, g),
                            x_chunk(kq, ni),
                            start=(kq == 0),
                            stop=(kq == KQ - 1),
                        )
                    scan_and_store(g, ni, ps_g)

            # warm-MM flush store, late, on scalar's ring (anti-DCE)
            nc.scalar.dma_start(flush_e[:], flush_sb[:])

    nc.compile()
    return nc


def _get_nc():
    if "nc" not in _state:
        _state["nc"] = _build_nc()
    return _state["nc"]


def _shard_inputs(x_seq, a_diag, b_mat):
    x = np.asarray(x_seq, dtype=np.float32)
    a = np.asarray(a_diag, dtype=np.float32)
    b = np.asarray(b_mat, dtype=np.float32)
    bq = np.ascontiguousarray(b.astype(ml_dtypes.bfloat16))
    x_pad = np.concatenate([np.zeros((HALO, H), np.float32), x], axis=0)
    xT = np.ascontiguousarray(x_pad.T).astype(ml_dtypes.bfloat16)  # [H, T+HALO]
    # av block: av[p, g] = a[g*128 + p], tiled down all 8 kq row-blocks
    av = np.tile(
        np.ascontiguousarray(a.reshape(G, P).T).astype(ml_dtypes.bfloat16),
        (KQ, 1),
    )  # [H, G]
    in_maps = []
    for i in range(NC):
        slab = np.concatenate(
            [av, xT[:, i * T_LOC : i * T_LOC + W]], axis=1
        )  # [H, AV + W]
        in_maps.append({"xt": np.ascontiguousarray(slab), "b": bq})
    return in_maps


def kernel(x_seq, a_diag, b_mat):
    from concourse.bass_utils import run_bass_kernel_spmd

    nc = _get_nc()
    in_maps = _shard_inputs(x_seq, a_diag, b_mat)
    res = run_bass_kernel_spmd(nc, in_maps, list(range(NC)))
    _state["last_result"] = res
    out = np.concatenate(
        [
            np.asarray(res.results[i]["out"]).astype(np.float32).T
            for i in range(NC)
        ],
        axis=0,
    )
    return out
